# revision 38
# baseline (speedup 1.0000x reference)
"""Trainium2 Bass kernel for nn_CPDist.

Math: with a = exp(h_last @ W.T + b).reshape(B, H, V, R), the reference
computes p_tilde[b,i,j] = sum_r a[b,0,i,r]*a[b,1,j,r], then
  p_eval[b]     = p_tilde[b, p0, p1]
  norm_const[b] = sum_ij p_tilde[b,i,j]
Both factorize over the rank dim, so the (B,V,V) slab is never needed:
  norm_const[b] = sum_r (sum_i a[b,0,i,r]) * (sum_j a[b,1,j,r])
  p_eval[b]     = sum_r a[b,0,p0,r] * a[b,1,p1,r]
The dominant cost is the (B=8, D=1024) x (D, V*R*H=131072) matmul + exp —
HBM-bound on streaming the weight matrix, shipped as scaled fp8.

Sharding: vocab dim V split across 8 cores (512 vocab rows each, for both
horizon slots). Each core streams its (1024, 16384) transposed fp8 weight
slab through the PE array (DoubleRow: 2 rows/cycle) against a stationary
h^T; exp on the scalar engine, whose accum_out yields per-(h,r) vocab-sum
partials. The 256 gathered rows needed for p_eval ride inside the same
fp8 stream, 32 columns per core, exp'd raw (no accum); the host picks the
own-batch entries. Host combines the (8, 32+) per-core partials.

Tail: the last 512-col chunk streams as narrow sub-blocks that
accumulate raw logits into one shared PSUM strip exp'd once, so after
the final weight DMA only one short matmul chain plus a single
exp+accum (one ACT min-delay + one accumulator read) stand before the
output DMA.
"""

import os

import numpy as np

import concourse.bacc as bacc
import concourse.bass as bass
import concourse.mybir as mybir
import concourse.tile as tile

B, T, D = 8, 128, 1024
V, R, H = 4096, 16, 2
NCORES = 8
VSH = V // NCORES            # vocab rows per core (512)
CHUNK = VSH                  # columns per full (h, r) chunk
NCHUNK = H * R               # 32 chunks of 512 columns per core
KT = D // 128                # 8 contraction tiles
NG_ALL = B * H * R           # 256 gathered columns for p_eval
NG_C = NG_ALL // NCORES      # 32 gathered columns per core

F32 = mybir.dt.float32
F32R = mybir.dt.float32r

_MM_NAME = os.environ.get("CPDIST_MM_DTYPE", "float8e4")
MM_DTYPE = getattr(mybir.dt, _MM_NAME)
# fp8 operands are pre-scaled into e4m3's sweet spot; the activation's scale
# argument undoes S*S on the logits before exp.
MM_SCALE = 1024.0 if MM_DTYPE == mybir.dt.float8e4 else 1.0
MM_ITEM = np.dtype(mybir.dt.np(MM_DTYPE)).itemsize

CPD = int(os.environ.get("CPDIST_CPD", "2"))          # chunks per weight DMA
HEAD = [int(x) for x in os.environ.get("CPDIST_HEAD", "1,1").split(",") if x]
# widths of the tail sub-blocks covering the last chunks of the stream
# (sum must be a multiple of CHUNK and each sub must stay inside one chunk).
# Every sub gets its own exp+accum except the last, whose raw logits land in
# a shared psum strip exp'd once at the very end — the only ACT work left
# after the final weight DMA.
_TS = os.environ.get("CPDIST_TAILSUB", "/128,128,128,64,64")
if "/" in _TS:
    _own, _strip = _TS.split("/")
else:
    _own, _strip = _TS, ""
OWNSUB = [int(x) for x in _own.split(",") if x]
STRIPSUB = [int(x) for x in _strip.split(",") if x]
if not STRIPSUB:
    OWNSUB, STRIPSUB = OWNSUB[:-1], OWNSUB[-1:]
TAILSUB = OWNSUB + STRIPSUB
STRIPW = sum(STRIPSUB)
# SWDGE prepared-scatter output (trigger_dma tail) deadlocks in the tile
# epilogue (the DMASW lane sem of a gen_mode=1 prep is never incremented),
# so the output leaves via a plain HWDGE DMA by default.
SCATTER = os.environ.get("CPDIST_SCATTER", "0") == "1"
PSBUFS = int(os.environ.get("CPDIST_PSBUFS", "4"))
WBUFS = int(os.environ.get("CPDIST_WBUFS", "6"))
SUBBUFS = int(os.environ.get("CPDIST_SUBBUFS", "0"))  # 0 = auto
DR = os.environ.get("CPDIST_DR",
                    "1" if MM_DTYPE == mybir.dt.float8e4 else "0") == "1"
# k-tiles of the last tail sub streamed early (must be even; 0 disables)
KSPLIT = int(os.environ.get("CPDIST_KSPLIT", "0"))
assert KSPLIT % 2 == 0 and 0 <= KSPLIT < KT

assert sum(TAILSUB) % CHUNK == 0
TC = sum(TAILSUB) // CHUNK   # chunks covered by the tail region
NSUB = len(TAILSUB)
assert STRIPW <= CHUNK       # the strip lives inside the final chunk
_cum = 0
for _w in TAILSUB:
    assert _cum // CHUNK == (_cum + _w - 1) // CHUNK, "sub straddles a chunk"
    _cum += _w
GC0 = (NCHUNK - TC) + len(OWNSUB) + 1   # first gather column in sg
NSC = GC0 + NG_C             # live output columns per core
NSC_PAD = 128                # padded so scatter rows are 512B (256B-aligned)
TOTAL_COLS = NCHUNK * CHUNK  # 16384 streamed s-columns
WT_COLS = KT * TOTAL_COLS

# pack0 (mm dtype) column layout: ht tiled | DoubleRow ht (padded to 16-col
# pairs so the dual-fp8 LDWEIGHTS pair stride is 16 bytes) | the p_eval
# gather block ([k][j] layout) | zero pad to a 512B partition row so the
# single consts DMA dodges the sub-512B descriptor penalty
P2_HT = 0
P2_DR = P2_HT + KT * B
P2_G = P2_DR + (KT // 2) * 32
P0_COLS = 512
assert P2_G + KT * NG_C <= P0_COLS

_cached = {}
_fast = {}
_last_results = None
_last_nc = None


def _round_fp32r(x):
    u = x.view(np.uint32)
    u = (u + np.uint32(0x7FF) + ((u >> np.uint32(12)) & np.uint32(1))) & np.uint32(
        0xFFFFF000
    )
    return u.view(np.float32)


def _to_mm(x, scale=1.0):
    x = np.ascontiguousarray(x, dtype=np.float32)
    if scale != 1.0:
        x = x * np.float32(scale)
    if MM_DTYPE == mybir.dt.float32r:
        return _round_fp32r(x)
    return x.astype(mybir.dt.np(MM_DTYPE))


def _tile_k(x):
    # (D, N) -> (128, KT*N) with column blocks per contraction tile
    n = x.shape[1]
    return np.ascontiguousarray(
        x.reshape(KT, 128, n).transpose(1, 0, 2).reshape(128, KT * n)
    )


def _plan():
    """Column blocks of the per-core weight stream, in stream order, and
    their grouping into DMAs. The gather block is not part of the stream —
    it rides in pack0. Blocks carry per-partition element offsets because
    the k-split pieces of the final strip are partial-K."""
    blocks = []
    off = 0
    NFC = NCHUNK - TC            # chunks streamed whole

    def add(w, k0, k1, sgcol, tail_off, bias_col, tr=False, src_col=None):
        nonlocal off
        blocks.append({"w": w, "k0": k0, "k1": k1, "off": off,
                       "nel": (k1 - k0) * w, "sgcol": sgcol,
                       "tail": tail_off, "bias_col": bias_col, "tr": tr,
                       "src_col": src_col if src_col is not None else bias_col})
        off += (k1 - k0) * w

    for ch in range(NFC):
        add(CHUNK, 0, KT, ch, None, ch * CHUNK)
    tcols0 = NFC * CHUNK
    tails, toff = [], 0
    for w in TAILSUB:
        tails.append((w, toff))
        toff += w
    strip0 = sum(OWNSUB)         # tail-region col where the strip starts
    ksp = KSPLIT if len(STRIPSUB) > 0 else 0
    scol_chunk = list(range(NFC))
    if ksp:
        # stream the final strip piece's first k-tiles ahead of the other
        # tail subs so only one DoubleRow matmul follows the last weight DMA
        wL, oL = tails[-1]
        add(wL, 0, ksp, None, oL - strip0, tcols0 + oL, tr=True,
            src_col=tcols0 + oL)
    for i, (w, to) in enumerate(tails[:len(OWNSUB)]):
        add(w, 0, KT, NFC + i, None, tcols0 + to, tr=True)
        scol_chunk.append(NFC + to // CHUNK)
    for (w, to) in (tails[len(OWNSUB):-1] if ksp else tails[len(OWNSUB):]):
        add(w, 0, KT, None, to - strip0, tcols0 + to, tr=True,
            src_col=tcols0 + to)
    if ksp:
        wL, oL = tails[-1]
        add(wL, ksp, KT, None, oL - strip0, None, tr=True,
            src_col=tcols0 + oL)
    scol_chunk.append(NFC + tails[-1][1] // CHUNK)   # the strip column
    assert off == WT_COLS
    assert len(scol_chunk) == GC0

    groups = []
    i = 0
    rem = NFC - sum(HEAD)
    chunk_groups = list(HEAD) + [CPD] * (rem // CPD) + \
        ([rem % CPD] if rem % CPD else [])
    for n in chunk_groups:
        groups.append(list(range(i, i + n)))
        i += n
    ntail = len(blocks) - i
    if ksp:
        # E rides in one DMA with the first full tail sub (>=512B elem runs)
        groups.append([i, i + 1])
        i += 2
        ntail -= 2
    for _ in range(ntail):
        groups.append([i])
        i += 1
    assert i == len(blocks)
    tail_pad = max(
        blocks[g[-1]]["off"] + blocks[g[-1]]["nel"] - blocks[g[0]]["off"]
        for g in groups if blocks[g[0]]["tr"])
    return blocks, groups, tail_pad, scol_chunk


def _build_nc(mm_dtype, use_bias):
    nc = bacc.Bacc("TRN2", target_bir_lowering=False)
    pack0 = nc.dram_tensor("pack0", (128, P0_COLS), mm_dtype, kind="ExternalInput")
    wt = nc.dram_tensor("wt", (128, WT_COLS), mm_dtype, kind="ExternalInput")
    if SCATTER:
        sidx_t = nc.dram_tensor("sidx", (16, 1), mybir.dt.int16,
                                kind="ExternalInput")
    if use_bias:
        bias_m = nc.dram_tensor("bias_m", (1, TOTAL_COLS + NG_C + B), F32R,
                                kind="ExternalInput")
    sg_out = nc.dram_tensor("sg_out", (B, NSC_PAD), F32, kind="ExternalOutput")

    blocks, groups, tail_pad, _scm = _plan()
    n_tail_groups = sum(1 for g in groups if blocks[g[0]]["tr"])
    subbufs = SUBBUFS if SUBBUFS else n_tail_groups + 1

    with tile.TileContext(nc) as tc:
        with (
            tc.tile_pool(name="consts", bufs=1) as consts,
            tc.tile_pool(name="wpool", bufs=WBUFS) as wpool,
            tc.tile_pool(name="spool", bufs=subbufs) as spool,
            tc.tile_pool(name="pspool", bufs=PSBUFS, space="PSUM") as pspool,
            tc.tile_pool(name="tailpool", bufs=1, space="PSUM") as tailpool,
            tc.tile_pool(name="epool", bufs=3) as epool,
        ):
            def issue_group(gi):
                blks = groups[gi]
                e0 = blocks[blks[0]]["off"]
                e1 = blocks[blks[-1]]["off"] + blocks[blks[-1]]["nel"]
                if blocks[blks[0]]["tr"]:
                    wtile = spool.tile([128, e1 - e0], mm_dtype,
                                       padded_shape=[128, tail_pad],
                                       name=f"ws_{gi}", tag="ws")
                else:
                    wtile = wpool.tile([128, e1 - e0], mm_dtype,
                                       padded_shape=[128, KT * CHUNK * CPD],
                                       name=f"w_{gi}", tag="w")
                nc.sync.dma_start(out=wtile[:], in_=wt[:, e0:e1])
                return wtile, e0

            PRE_ISSUE = min(3, len(groups))
            pre = [issue_group(gi) for gi in range(PRE_ISSUE)]

            p0_sb = consts.tile([128, P0_COLS], mm_dtype)
            nc.sync.dma_start(out=p0_sb[:], in_=pack0[:])
            if use_bias:
                bias_sb = consts.tile([1, TOTAL_COLS + NG_C + B], F32R)
                nc.sync.dma_start(out=bias_sb[:], in_=bias_m[:])
                ones_r = bias_sb[0:1, TOTAL_COLS + NG_C:TOTAL_COLS + NG_C + B]
            if SCATTER:
                sidx_sb = consts.tile([16, 1], mybir.dt.int16)
                nc.sync.dma_start(out=sidx_sb[:], in_=sidx_t[:])

            sg = consts.tile([128, NSC_PAD], F32)
            if SCATTER:
                # rows 8..127 are scatter-skipped (idx -1) but still read by
                # the DMA; zero them so nothing downstream sees garbage
                nc.gpsimd.memset(sg[:], 0.0)
                dma_sem = nc.alloc_semaphore("sg_dma")
                nc.gpsimd.dma_scatter_add(
                    sg_out[:],
                    sg[:].rearrange("p (a e) -> p a e", a=1),
                    sidx_sb[:],
                    16, 16, NSC_PAD,
                    prepare_only=True,
                    sem=dma_sem,
                )

            ps_tail = tailpool.tile([B, STRIPW], F32)

            def ht_k(k):
                return p0_sb[:, P2_HT + k * B:P2_HT + (k + 1) * B]

            def emit_block(rhs_base, boff, blk, exp_out, accum_out):
                w, k0, k1 = blk["w"], blk["k0"], blk["k1"]
                if blk["tail"] is not None:
                    # tail sub-block: accumulate raw logits into the shared
                    # psum strip, exp'd once at the end
                    ps_ap = ps_tail[0:B, blk["tail"]:blk["tail"] + w]
                else:
                    ps = pspool.tile([B, w], F32, tag="ps",
                                     padded_shape=[B, CHUNK])
                    ps_ap = ps[:]
                started = False
                if use_bias and blk["bias_col"] is not None:
                    # bias matmul first: reads only long-resident tiles,
                    # absorbing the psum-slot-free wait
                    bc0 = blk["bias_col"]
                    nc.tensor.matmul(
                        ps_ap,
                        lhsT=ones_r,
                        rhs=bias_sb[:, bc0:bc0 + w],
                        start=(k0 == 0),
                        stop=False,
                    )
                    started = True
                if DR:
                    for k2 in range(k0 // 2, k1 // 2):
                        rel = 2 * k2 - k0
                        nc.tensor.matmul(
                            ps_ap,
                            lhsT=p0_sb[:, P2_DR + k2 * 32:P2_DR + (k2 + 1) * 32]
                                .rearrange("p (i m) -> p i m", i=2)[:, :, 0:B],
                            rhs=rhs_base[:, boff + rel * w:
                                         boff + (rel + 2) * w]
                                .rearrange("p (i n) -> p i n", i=2),
                            start=(not started and k0 == 0 and k2 == 0),
                            stop=(k1 == KT and k2 == KT // 2 - 1),
                            perf_mode=mybir.MatmulPerfMode.DoubleRow,
                        )
                else:
                    for k in range(k0, k1):
                        nc.tensor.matmul(
                            ps_ap,
                            lhsT=ht_k(k),
                            rhs=rhs_base[:, boff + (k - k0) * w:
                                         boff + (k - k0 + 1) * w],
                            start=(not started and k == 0),
                            stop=(k1 == KT and k == KT - 1),
                        )
                if blk["tail"] is not None:
                    return
                if exp_out is None:
                    e_tile = epool.tile([B, w], F32, tag="e", name="e_tile",
                                        padded_shape=[B, CHUNK])
                    exp_out = e_tile[:]
                kw = {} if accum_out is None else {"accum_out": accum_out}
                nc.scalar.activation(
                    exp_out,
                    ps_ap,
                    mybir.ActivationFunctionType.Exp,
                    scale=1.0 / (MM_SCALE * MM_SCALE),
                    **kw,
                )

            gblk = {"w": NG_C, "k0": 0, "k1": KT, "tail": None,
                    "sgcol": None, "bias_col": TOTAL_COLS}
            for gi in range(len(groups)):
                wtile, goff = pre[gi] if gi < PRE_ISSUE else issue_group(gi)
                for bi in groups[gi]:
                    blk = blocks[bi]
                    acc = (None if blk["sgcol"] is None else
                           sg[0:B, blk["sgcol"]:blk["sgcol"] + 1])
                    emit_block(wtile, blk["off"] - goff, blk, None, acc)
                if gi == 0:
                    # p_eval gather block: rides in pack0, raw exp into sg.
                    # Emitted after group 0 so its ACT exp (which waits on
                    # pack0) doesn't head-of-line block the chunk exps.
                    emit_block(p0_sb, P2_G, gblk,
                               sg[0:B, GC0:GC0 + NG_C], None)

            # final strip: one exp+accum, the only ACT work left after the
            # last weight DMA
            e_tail = epool.tile([B, STRIPW], F32, name="e_tail", tag="e",
                                padded_shape=[B, CHUNK])
            nc.scalar.activation(
                e_tail[:],
                ps_tail[:],
                mybir.ActivationFunctionType.Exp,
                scale=1.0 / (MM_SCALE * MM_SCALE),
                accum_out=sg[0:B, GC0 - 1:GC0],
            )
            if SCATTER:
                nc.gpsimd.trigger_dma(count=None)
            else:
                nc.sync.dma_start(out=sg_out[:], in_=sg[0:B, :])
    nc.compile()
    return nc


def _get_nc(use_bias=False):
    key = (str(MM_DTYPE), CPD, WBUFS, DR, PSBUFS, SCATTER,
           tuple(HEAD), tuple(TAILSUB), KSPLIT, use_bias)
    if key not in _cached:
        _cached[key] = _build_nc(MM_DTYPE, use_bias)
    return _cached[key]


def _prep_core_inputs(W, bias_vec, points, ht, use_bias):
    W4 = W.reshape(H, V, R, D)
    b3 = bias_vec.reshape(H, V, R)
    blocks, _, _, _ = _plan()

    # gathered rows for p_eval: global column order (h, r, b)
    rows = np.empty((NG_ALL,), np.int64)
    for h in range(H):
        for r in range(R):
            for b in range(B):
                rows[(h * R + r) * B + b] = h * V * R + int(points[b, h]) * R + r
    wg_all = W[rows, :]                    # (NG_ALL, D)
    bg_all = bias_vec[rows]

    pack0 = np.zeros((128, P0_COLS), np.float32)
    ht_t = _tile_k(ht.astype(np.float32)) * np.float32(MM_SCALE)  # (128, KT*B)
    pack0[:, P2_HT:P2_HT + KT * B] = ht_t
    for k2 in range(KT // 2):
        for i in range(2):
            k = 2 * k2 + i
            pack0[:, P2_DR + k2 * 32 + i * 16:P2_DR + k2 * 32 + i * 16 + B] = \
                ht_t[:, k * B:(k + 1) * B]

    sidx = np.full((16, 1), -1, np.int16)
    sidx[:B, 0] = np.arange(B, dtype=np.int16)

    common = {}
    if SCATTER:
        common["sidx"] = sidx

    in_maps = []
    for c in range(NCORES):
        sl = slice(c * VSH, (c + 1) * VSH)
        # (h, v, r, k, p) -> (p, h, r, k, v): per-chunk [k][v] layout
        s5 = W4[:, sl, :, :].reshape(H, VSH, R, KT, 128)
        slab = s5.transpose(4, 0, 2, 3, 1).reshape(128, NCHUNK, KT, VSH)
        g_j = np.arange(c * NG_C, (c + 1) * NG_C)
        gblk = wg_all[g_j].reshape(NG_C, KT, 128).transpose(2, 1, 0)  # (p,k,j)

        p0 = pack0.copy()
        p0[:, P2_G:P2_G + KT * NG_C] = \
            gblk.reshape(128, KT * NG_C) * np.float32(MM_SCALE)

        wparts = []
        bc = np.ascontiguousarray(
            b3[:, sl, :].transpose(0, 2, 1)).reshape(NCHUNK, VSH)
        for blk in blocks:
            w, k0, k1 = blk["w"], blk["k0"], blk["k1"]
            if not blk["tr"]:
                ch = blk["sgcol"]
                assert w == CHUNK and (k0, k1) == (0, KT)
                wparts.append(slab[:, ch].reshape(128, KT * CHUNK))
            else:
                # (piece of a) tail sub-block
                ch, v0 = blk["src_col"] // CHUNK, blk["src_col"] % CHUNK
                wparts.append(np.ascontiguousarray(
                    slab[:, ch, k0:k1, v0:v0 + w]).reshape(
                        128, (k1 - k0) * w))
        wt_full = _to_mm(np.concatenate(wparts, axis=1), MM_SCALE)
        assert wt_full.shape == (128, WT_COLS)
        m = {**common, "wt": wt_full, "pack0": _to_mm(p0)}
        if use_bias:
            # bias columns in natural chunk-major order (bias_col indexes)
            bcr = np.empty((1, TOTAL_COLS + NG_C + B), np.float32)
            bcr[0, :TOTAL_COLS] = bc.reshape(-1) * np.float32(
                MM_SCALE * MM_SCALE)
            bcr[0, TOTAL_COLS:TOTAL_COLS + NG_C] = bg_all[g_j] * np.float32(
                MM_SCALE * MM_SCALE)
            bcr[0, TOTAL_COLS + NG_C:] = 1.0
            m["bias_m"] = _round_fp32r(np.ascontiguousarray(bcr))
        in_maps.append(m)
    return in_maps


def _build_fast(nc):
    """Cache a jitted executor for this nc so repeat kernel() calls skip
    retracing/recompiling (mirrors bass2jax.run_bass_via_pjrt)."""
    import jax
    from concourse import bass2jax
    from concourse.bass2jax import _bass_exec_p, partition_id_tensor
    from jax.experimental.shard_map import shard_map
    from jax.sharding import Mesh, PartitionSpec

    bass2jax.install_neuronx_cc_hook()
    partition_name = nc.partition_id_tensor.name if nc.partition_id_tensor else None
    in_names, out_names, out_avals, zero_outs = [], [], [], []
    for alloc in nc.m.functions[0].allocations:
        if not isinstance(alloc, mybir.MemoryLocationSet):
            continue
        name = alloc.memorylocations[0].name
        if alloc.kind == "ExternalInput":
            if name != partition_name:
                in_names.append(name)
        elif alloc.kind == "ExternalOutput":
            out_names.append(name)
            shape = tuple(alloc.tensor_shape)
            dtype = mybir.dt.np(alloc.dtype)
            out_avals.append(jax.core.ShapedArray(shape, dtype))
            zero_outs.append(np.zeros(shape, dtype))
    n_params = len(in_names)
    all_in = list(in_names) + list(out_names)
    if partition_name is not None:
        all_in.append(partition_name)

    def _body(*args):
        ops = list(args)
        if partition_name is not None:
            ops.append(partition_id_tensor())
        return tuple(
            _bass_exec_p.bind(
                *ops,
                out_avals=tuple(out_avals),
                in_names=tuple(all_in),
                out_names=tuple(out_names),
                lowering_input_output_aliases=(),
                sim_require_finite=True,
                sim_require_nnan=True,
                nc=nc,
            )
        )

    devices = jax.devices()[:NCORES]
    mesh = Mesh(np.asarray(devices), ("core",))
    spec = PartitionSpec("core")
    fn = jax.jit(
        shard_map(
            _body, mesh=mesh,
            in_specs=(spec,) * (n_params + len(out_names)),
            out_specs=(spec,) * len(out_names), check_rep=False,
        ),
        keep_unused=True,
    )
    _fast[id(nc)] = (fn, in_names, out_names, out_avals, zero_outs, mesh, spec)


def _run_cached(nc, in_maps):
    fn, in_names, out_names, out_avals, zero_outs, mesh, spec = _fast[id(nc)]
    concat_in = [
        np.concatenate([np.asarray(in_maps[c][nm]) for c in range(NCORES)], axis=0)
        for nm in in_names
    ]
    concat_zero = [
        np.zeros((NCORES * z.shape[0], *z.shape[1:]), z.dtype) for z in zero_outs
    ]
    outs = fn(*concat_in, *concat_zero)
    return [
        {
            nm: np.asarray(outs[i]).reshape(NCORES, *out_avals[i].shape)[c]
            for i, nm in enumerate(out_names)
        }
        for c in range(NCORES)
    ]


def kernel(last_hidden_state, param_w, param_b, points):
    global _last_results, _last_nc
    from concourse.bass_utils import run_bass_kernel_spmd

    lhs = np.asarray(last_hidden_state, dtype=np.float32)
    W = np.ascontiguousarray(np.asarray(param_w, dtype=np.float32))
    bias_vec = np.asarray(param_b, dtype=np.float32)
    pts = np.asarray(points)

    use_bias = bool(np.any(bias_vec))
    ht = np.ascontiguousarray(lhs[:, -1, :].T)  # (D, B)
    in_maps = _prep_core_inputs(W, bias_vec, pts, ht, use_bias)

    nc = _get_nc(use_bias=use_bias)
    _last_nc = nc
    if id(nc) in _fast:
        results = _run_cached(nc, in_maps)
    else:
        res = run_bass_kernel_spmd(nc, in_maps, core_ids=list(range(NCORES)))
        _last_results = res
        results = res.results
        _build_fast(nc)

    _, _, _, scol_chunk = _plan()
    sacc = np.zeros((B, GC0), np.float64)
    for r_ in results:
        sacc += r_["sg_out"][:, :GC0].astype(np.float64)
    s32 = np.zeros((B, NCHUNK), np.float64)
    for col, ch in enumerate(scol_chunk):
        s32[:, ch] += sacc[:, col]
    s0, s1 = s32[:, :R], s32[:, R:]
    norm_const = (s0 * s1).sum(axis=1)

    g_all = np.stack(
        [r_["sg_out"][:, GC0:GC0 + NG_C].astype(np.float64) for r_ in results],
        axis=1,
    ).reshape(B, NG_ALL)
    bi = np.arange(B)[:, None]
    ri = np.arange(R)[None, :]
    g0 = g_all[bi, ri * B + bi]
    g1 = g_all[bi, (R + ri) * B + bi]
    p_eval = (g0 * g1).sum(axis=1)
    return p_eval.astype(np.float32), norm_const.astype(np.float32)


# revision 41
# speedup vs baseline: 1.0006x; 1.0006x over previous
"""Trainium2 Bass kernel for nn_CPDist.

Math: with a = exp(h_last @ W.T + b).reshape(B, H, V, R), the reference
computes p_tilde[b,i,j] = sum_r a[b,0,i,r]*a[b,1,j,r], then
  p_eval[b]     = p_tilde[b, p0, p1]
  norm_const[b] = sum_ij p_tilde[b,i,j]
Both factorize over the rank dim, so the (B,V,V) slab is never needed:
  norm_const[b] = sum_r (sum_i a[b,0,i,r]) * (sum_j a[b,1,j,r])
  p_eval[b]     = sum_r a[b,0,p0,r] * a[b,1,p1,r]
The dominant cost is the (B=8, D=1024) x (D, V*R*H=131072) matmul + exp —
HBM-bound on streaming the weight matrix, shipped as scaled fp8.

Sharding: vocab dim V split across 8 cores (512 vocab rows each, for both
horizon slots). Each core streams its (1024, 16384) transposed fp8 weight
slab through the PE array (DoubleRow: 2 rows/cycle) against a stationary
h^T; exp on the scalar engine, whose accum_out yields per-(h,r) vocab-sum
partials. The 256 gathered rows needed for p_eval ride inside the same
fp8 stream, 32 columns per core, exp'd raw (no accum); the host picks the
own-batch entries. Host combines the (8, 32+) per-core partials.

Tail: the last 512-col chunk streams as narrow sub-blocks that
accumulate raw logits into one shared PSUM strip exp'd once, so after
the final weight DMA only one short matmul chain plus a single
exp+accum (one ACT min-delay + one accumulator read) stand before the
output DMA.
"""

import os

import numpy as np

import concourse.bacc as bacc
import concourse.bass as bass
import concourse.mybir as mybir
import concourse.tile as tile

B, T, D = 8, 128, 1024
V, R, H = 4096, 16, 2
NCORES = 8
VSH = V // NCORES            # vocab rows per core (512)
CHUNK = VSH                  # columns per full (h, r) chunk
NCHUNK = H * R               # 32 chunks of 512 columns per core
KT = D // 128                # 8 contraction tiles
NG_ALL = B * H * R           # 256 gathered columns for p_eval
NG_C = NG_ALL // NCORES      # 32 gathered columns per core

F32 = mybir.dt.float32
F32R = mybir.dt.float32r

_MM_NAME = os.environ.get("CPDIST_MM_DTYPE", "float8e4")
MM_DTYPE = getattr(mybir.dt, _MM_NAME)
# fp8 operands are pre-scaled into e4m3's sweet spot; the activation's scale
# argument undoes S*S on the logits before exp.
MM_SCALE = 1024.0 if MM_DTYPE == mybir.dt.float8e4 else 1.0
MM_ITEM = np.dtype(mybir.dt.np(MM_DTYPE)).itemsize

CPD = int(os.environ.get("CPDIST_CPD", "2"))          # chunks per weight DMA
HEAD = [int(x) for x in os.environ.get("CPDIST_HEAD", "1,1").split(",") if x]
# widths of the tail sub-blocks covering the last chunks of the stream
# (sum must be a multiple of CHUNK and each sub must stay inside one chunk).
# Every sub gets its own exp+accum except the last, whose raw logits land in
# a shared psum strip exp'd once at the very end — the only ACT work left
# after the final weight DMA.
_TS = os.environ.get("CPDIST_TAILSUB", "/128,128,128,64,64")
if "/" in _TS:
    _own, _strip = _TS.split("/")
else:
    _own, _strip = _TS, ""
OWNSUB = [int(x) for x in _own.split(",") if x]
STRIPSUB = [int(x) for x in _strip.split(",") if x]
if not STRIPSUB:
    OWNSUB, STRIPSUB = OWNSUB[:-1], OWNSUB[-1:]
TAILSUB = OWNSUB + STRIPSUB
STRIPW = sum(STRIPSUB)
# SWDGE prepared-scatter output (trigger_dma tail) deadlocks in the tile
# epilogue (the DMASW lane sem of a gen_mode=1 prep is never incremented),
# so the output leaves via a plain HWDGE DMA by default.
SCATTER = os.environ.get("CPDIST_SCATTER", "0") == "1"
PSBUFS = int(os.environ.get("CPDIST_PSBUFS", "4"))
WBUFS = int(os.environ.get("CPDIST_WBUFS", "6"))
SUBBUFS = int(os.environ.get("CPDIST_SUBBUFS", "0"))  # 0 = auto
DR = os.environ.get("CPDIST_DR",
                    "1" if MM_DTYPE == mybir.dt.float8e4 else "0") == "1"
# k-tiles of the last tail sub streamed early (must be even; 0 disables)
KSPLIT = int(os.environ.get("CPDIST_KSPLIT", "0"))
# chunk index after which the own-accum tail subs are spliced into the stream
OWNPOS = int(os.environ.get("CPDIST_OWNPOS", "16"))
assert KSPLIT % 2 == 0 and 0 <= KSPLIT < KT

assert sum(TAILSUB) % CHUNK == 0
TC = sum(TAILSUB) // CHUNK   # chunks covered by the tail region
NSUB = len(TAILSUB)
assert STRIPW <= CHUNK       # the strip lives inside the final chunk
_cum = 0
for _w in TAILSUB:
    assert _cum // CHUNK == (_cum + _w - 1) // CHUNK, "sub straddles a chunk"
    _cum += _w
GC0 = (NCHUNK - TC) + len(OWNSUB)   # first gather column in sg
TEXP0 = GC0 + NG_C           # raw tail-strip exps start here (no accum-read
                             # on the critical path; host sums them)
NSC = TEXP0 + STRIPW         # live output columns per core
NSC_PAD = (NSC + 127) // 128 * 128   # keep rows a multiple of 512B
TOTAL_COLS = NCHUNK * CHUNK  # 16384 streamed s-columns
WT_COLS = KT * TOTAL_COLS

# pack0 (mm dtype) column layout: ht tiled | DoubleRow ht (padded to 16-col
# pairs so the dual-fp8 LDWEIGHTS pair stride is 16 bytes) | the p_eval
# gather block ([k][j] layout) | zero pad to a 512B partition row so the
# single consts DMA dodges the sub-512B descriptor penalty
P2_HT = 0
P2_DR = P2_HT + KT * B
P2_G = P2_DR + (KT // 2) * 32
P0_COLS = 512
assert P2_G + KT * NG_C <= P0_COLS

_cached = {}
_fast = {}
_last_results = None
_last_nc = None


def _round_fp32r(x):
    u = x.view(np.uint32)
    u = (u + np.uint32(0x7FF) + ((u >> np.uint32(12)) & np.uint32(1))) & np.uint32(
        0xFFFFF000
    )
    return u.view(np.float32)


def _to_mm(x, scale=1.0):
    x = np.ascontiguousarray(x, dtype=np.float32)
    if scale != 1.0:
        x = x * np.float32(scale)
    if MM_DTYPE == mybir.dt.float32r:
        return _round_fp32r(x)
    return x.astype(mybir.dt.np(MM_DTYPE))


def _tile_k(x):
    # (D, N) -> (128, KT*N) with column blocks per contraction tile
    n = x.shape[1]
    return np.ascontiguousarray(
        x.reshape(KT, 128, n).transpose(1, 0, 2).reshape(128, KT * n)
    )


def _plan():
    """Column blocks of the per-core weight stream, in stream order, and
    their grouping into DMAs. The gather block is not part of the stream —
    it rides in pack0. Blocks carry per-partition element offsets because
    the k-split pieces of the final strip are partial-K.

    Own-accum tail subs are spliced into the middle of the stream (after
    chunk OWNPOS) so their full drain chains hide under later streaming;
    only the strip subs sit at the very end."""
    blocks = []
    groups = []
    off = 0
    NFC = NCHUNK - TC            # chunks streamed whole

    def add(w, k0, k1, sgcol, tail_off, bias_col, tr=False, src_col=None):
        nonlocal off
        blocks.append({"w": w, "k0": k0, "k1": k1, "off": off,
                       "nel": (k1 - k0) * w, "sgcol": sgcol,
                       "tail": tail_off, "bias_col": bias_col, "tr": tr,
                       "src_col": src_col if src_col is not None else bias_col})
        off += (k1 - k0) * w
        return len(blocks) - 1

    tcols0 = NFC * CHUNK
    tails, toff = [], 0
    for w in TAILSUB:
        tails.append((w, toff))
        toff += w
    strip0 = sum(OWNSUB)         # tail-region col where the strip starts
    ksp = KSPLIT if len(STRIPSUB) > 0 else 0
    scol_chunk = list(range(NFC))

    # chunk groups (HEAD then CPD) with own-accum subs spliced in after
    # chunk OWNPOS, then [E], strip subs, [L]
    rem = NFC - sum(HEAD)
    chunk_groups = list(HEAD) + [CPD] * (rem // CPD) + \
        ([rem % CPD] if rem % CPD else [])
    ch = 0
    own_done = len(OWNSUB) == 0
    for gi, n in enumerate(chunk_groups):
        groups.append([add(CHUNK, 0, KT, c, None, c * CHUNK)
                       for c in range(ch, ch + n)])
        ch += n
        if ch > OWNPOS and not own_done:
            own_done = True
            for i, (w, to) in enumerate(tails[:len(OWNSUB)]):
                groups.append([add(w, 0, KT, NFC + i, None, tcols0 + to,
                                   tr=True)])
                scol_chunk.append(NFC + to // CHUNK)
    assert ch == NFC and own_done
    if ksp:
        # stream the final strip piece's first k-tiles ahead of the other
        # strip subs (merged into the last chunk group's DMA) so only one
        # DoubleRow matmul follows the last weight DMA
        wL, oL = tails[-1]
        groups[-1].append(add(wL, 0, ksp, None, oL - strip0, tcols0 + oL,
                              tr=True, src_col=tcols0 + oL))
    for (w, to) in (tails[len(OWNSUB):-1] if ksp else tails[len(OWNSUB):]):
        groups.append([add(w, 0, KT, None, to - strip0, tcols0 + to,
                           tr=True, src_col=tcols0 + to)])
    if ksp:
        wL, oL = tails[-1]
        groups.append([add(wL, ksp, KT, None, oL - strip0, None,
                           tr=True, src_col=tcols0 + oL)])
    assert off == WT_COLS
    assert len(scol_chunk) == GC0
    def gnel(g):
        return blocks[g[-1]]["off"] + blocks[g[-1]]["nel"] - blocks[g[0]]["off"]
    tail_pad = max(gnel(g) for g in groups if blocks[g[0]]["tr"])
    w_pad = max(gnel(g) for g in groups if not blocks[g[0]]["tr"])
    return blocks, groups, (w_pad, tail_pad), scol_chunk


def _build_nc(mm_dtype, use_bias):
    nc = bacc.Bacc("TRN2", target_bir_lowering=False)
    pack0 = nc.dram_tensor("pack0", (128, P0_COLS), mm_dtype, kind="ExternalInput")
    wt = nc.dram_tensor("wt", (128, WT_COLS), mm_dtype, kind="ExternalInput")
    if SCATTER:
        sidx_t = nc.dram_tensor("sidx", (16, 1), mybir.dt.int16,
                                kind="ExternalInput")
    if use_bias:
        bias_m = nc.dram_tensor("bias_m", (1, TOTAL_COLS + NG_C + B), F32R,
                                kind="ExternalInput")
    sg_out = nc.dram_tensor("sg_out", (B, NSC_PAD), F32, kind="ExternalOutput")

    blocks, groups, (w_pad, tail_pad), _scm = _plan()
    n_tail_groups = sum(1 for g in groups if blocks[g[0]]["tr"])
    subbufs = SUBBUFS if SUBBUFS else n_tail_groups + 1

    with tile.TileContext(nc) as tc:
        with (
            tc.tile_pool(name="consts", bufs=1) as consts,
            tc.tile_pool(name="wpool", bufs=WBUFS) as wpool,
            tc.tile_pool(name="spool", bufs=subbufs) as spool,
            tc.tile_pool(name="pspool", bufs=PSBUFS, space="PSUM") as pspool,
            tc.tile_pool(name="tailpool", bufs=1, space="PSUM") as tailpool,
            tc.tile_pool(name="epool", bufs=3) as epool,
        ):
            def issue_group(gi):
                blks = groups[gi]
                e0 = blocks[blks[0]]["off"]
                e1 = blocks[blks[-1]]["off"] + blocks[blks[-1]]["nel"]
                if blocks[blks[0]]["tr"]:
                    wtile = spool.tile([128, e1 - e0], mm_dtype,
                                       padded_shape=[128, tail_pad],
                                       name=f"ws_{gi}", tag="ws")
                else:
                    wtile = wpool.tile([128, e1 - e0], mm_dtype,
                                       padded_shape=[128, w_pad],
                                       name=f"w_{gi}", tag="w")
                nc.sync.dma_start(out=wtile[:], in_=wt[:, e0:e1])
                return wtile, e0

            PRE_ISSUE = min(3, len(groups))
            pre = [issue_group(gi) for gi in range(PRE_ISSUE)]

            p0_sb = consts.tile([128, P0_COLS], mm_dtype)
            nc.sync.dma_start(out=p0_sb[:], in_=pack0[:])
            if use_bias:
                bias_sb = consts.tile([1, TOTAL_COLS + NG_C + B], F32R)
                nc.sync.dma_start(out=bias_sb[:], in_=bias_m[:])
                ones_r = bias_sb[0:1, TOTAL_COLS + NG_C:TOTAL_COLS + NG_C + B]
            if SCATTER:
                sidx_sb = consts.tile([16, 1], mybir.dt.int16)
                nc.sync.dma_start(out=sidx_sb[:], in_=sidx_t[:])

            sg = consts.tile([128, NSC_PAD], F32)
            if SCATTER:
                # rows 8..127 are scatter-skipped (idx -1) but still read by
                # the DMA; zero them so nothing downstream sees garbage
                nc.gpsimd.memset(sg[:], 0.0)
                dma_sem = nc.alloc_semaphore("sg_dma")
                nc.gpsimd.dma_scatter_add(
                    sg_out[:],
                    sg[:].rearrange("p (a e) -> p a e", a=1),
                    sidx_sb[:],
                    16, 16, NSC_PAD,
                    prepare_only=True,
                    sem=dma_sem,
                )

            ps_tail = tailpool.tile([B, STRIPW], F32)

            def ht_k(k):
                return p0_sb[:, P2_HT + k * B:P2_HT + (k + 1) * B]

            def emit_block(rhs_base, boff, blk, exp_out, accum_out):
                w, k0, k1 = blk["w"], blk["k0"], blk["k1"]
                if blk["tail"] is not None:
                    # tail sub-block: accumulate raw logits into the shared
                    # psum strip, exp'd once at the end
                    ps_ap = ps_tail[0:B, blk["tail"]:blk["tail"] + w]
                else:
                    ps = pspool.tile([B, w], F32, tag="ps",
                                     padded_shape=[B, CHUNK])
                    ps_ap = ps[:]
                started = False
                if use_bias and blk["bias_col"] is not None:
                    # bias matmul first: reads only long-resident tiles,
                    # absorbing the psum-slot-free wait
                    bc0 = blk["bias_col"]
                    nc.tensor.matmul(
                        ps_ap,
                        lhsT=ones_r,
                        rhs=bias_sb[:, bc0:bc0 + w],
                        start=(k0 == 0),
                        stop=False,
                    )
                    started = True
                if DR:
                    for k2 in range(k0 // 2, k1 // 2):
                        rel = 2 * k2 - k0
                        nc.tensor.matmul(
                            ps_ap,
                            lhsT=p0_sb[:, P2_DR + k2 * 32:P2_DR + (k2 + 1) * 32]
                                .rearrange("p (i m) -> p i m", i=2)[:, :, 0:B],
                            rhs=rhs_base[:, boff + rel * w:
                                         boff + (rel + 2) * w]
                                .rearrange("p (i n) -> p i n", i=2),
                            start=(not started and k0 == 0 and k2 == 0),
                            stop=(k1 == KT and k2 == KT // 2 - 1),
                            perf_mode=mybir.MatmulPerfMode.DoubleRow,
                        )
                else:
                    for k in range(k0, k1):
                        nc.tensor.matmul(
                            ps_ap,
                            lhsT=ht_k(k),
                            rhs=rhs_base[:, boff + (k - k0) * w:
                                         boff + (k - k0 + 1) * w],
                            start=(not started and k == 0),
                            stop=(k1 == KT and k == KT - 1),
                        )
                if blk["tail"] is not None:
                    return
                if exp_out is None:
                    e_tile = epool.tile([B, w], F32, tag="e", name="e_tile",
                                        padded_shape=[B, CHUNK])
                    exp_out = e_tile[:]
                kw = {} if accum_out is None else {"accum_out": accum_out}
                nc.scalar.activation(
                    exp_out,
                    ps_ap,
                    mybir.ActivationFunctionType.Exp,
                    scale=1.0 / (MM_SCALE * MM_SCALE),
                    **kw,
                )

            gblk = {"w": NG_C, "k0": 0, "k1": KT, "tail": None,
                    "sgcol": None, "bias_col": TOTAL_COLS}
            for gi in range(len(groups)):
                wtile, goff = pre[gi] if gi < PRE_ISSUE else issue_group(gi)
                for bi in groups[gi]:
                    blk = blocks[bi]
                    acc = (None if blk["sgcol"] is None else
                           sg[0:B, blk["sgcol"]:blk["sgcol"] + 1])
                    emit_block(wtile, blk["off"] - goff, blk, None, acc)
                if gi == 0:
                    # p_eval gather block: rides in pack0, raw exp into sg.
                    # Emitted after group 0 so its ACT exp (which waits on
                    # pack0) doesn't head-of-line block the chunk exps.
                    emit_block(p0_sb, P2_G, gblk,
                               sg[0:B, GC0:GC0 + NG_C], None)

            # final strip: a single exp writing raw values straight into the
            # output tile — no accumulator read on the critical path, the
            # host sums the strip columns
            nc.scalar.activation(
                sg[0:B, TEXP0:TEXP0 + STRIPW],
                ps_tail[:],
                mybir.ActivationFunctionType.Exp,
                scale=1.0 / (MM_SCALE * MM_SCALE),
            )
            if SCATTER:
                nc.gpsimd.trigger_dma(count=None)
            else:
                nc.sync.dma_start(out=sg_out[:], in_=sg[0:B, :])
    nc.compile()
    return nc


def _get_nc(use_bias=False):
    key = (str(MM_DTYPE), CPD, WBUFS, DR, PSBUFS, SCATTER,
           tuple(HEAD), tuple(TAILSUB), KSPLIT, use_bias)
    if key not in _cached:
        _cached[key] = _build_nc(MM_DTYPE, use_bias)
    return _cached[key]


def _prep_core_inputs(W, bias_vec, points, ht, use_bias):
    W4 = W.reshape(H, V, R, D)
    b3 = bias_vec.reshape(H, V, R)
    blocks, _, _, _ = _plan()

    # gathered rows for p_eval: global column order (h, r, b)
    rows = np.empty((NG_ALL,), np.int64)
    for h in range(H):
        for r in range(R):
            for b in range(B):
                rows[(h * R + r) * B + b] = h * V * R + int(points[b, h]) * R + r
    wg_all = W[rows, :]                    # (NG_ALL, D)
    bg_all = bias_vec[rows]

    pack0 = np.zeros((128, P0_COLS), np.float32)
    ht_t = _tile_k(ht.astype(np.float32)) * np.float32(MM_SCALE)  # (128, KT*B)
    pack0[:, P2_HT:P2_HT + KT * B] = ht_t
    for k2 in range(KT // 2):
        for i in range(2):
            k = 2 * k2 + i
            pack0[:, P2_DR + k2 * 32 + i * 16:P2_DR + k2 * 32 + i * 16 + B] = \
                ht_t[:, k * B:(k + 1) * B]

    sidx = np.full((16, 1), -1, np.int16)
    sidx[:B, 0] = np.arange(B, dtype=np.int16)

    common = {}
    if SCATTER:
        common["sidx"] = sidx

    in_maps = []
    for c in range(NCORES):
        sl = slice(c * VSH, (c + 1) * VSH)
        # (h, v, r, k, p) -> (p, h, r, k, v): per-chunk [k][v] layout
        s5 = W4[:, sl, :, :].reshape(H, VSH, R, KT, 128)
        slab = s5.transpose(4, 0, 2, 3, 1).reshape(128, NCHUNK, KT, VSH)
        g_j = np.arange(c * NG_C, (c + 1) * NG_C)
        gblk = wg_all[g_j].reshape(NG_C, KT, 128).transpose(2, 1, 0)  # (p,k,j)

        p0 = pack0.copy()
        p0[:, P2_G:P2_G + KT * NG_C] = \
            gblk.reshape(128, KT * NG_C) * np.float32(MM_SCALE)

        wparts = []
        bc = np.ascontiguousarray(
            b3[:, sl, :].transpose(0, 2, 1)).reshape(NCHUNK, VSH)
        for blk in blocks:
            w, k0, k1 = blk["w"], blk["k0"], blk["k1"]
            if not blk["tr"]:
                ch = blk["sgcol"]
                assert w == CHUNK and (k0, k1) == (0, KT)
                wparts.append(slab[:, ch].reshape(128, KT * CHUNK))
            else:
                # (piece of a) tail sub-block
                ch, v0 = blk["src_col"] // CHUNK, blk["src_col"] % CHUNK
                wparts.append(np.ascontiguousarray(
                    slab[:, ch, k0:k1, v0:v0 + w]).reshape(
                        128, (k1 - k0) * w))
        wt_full = _to_mm(np.concatenate(wparts, axis=1), MM_SCALE)
        assert wt_full.shape == (128, WT_COLS)
        m = {**common, "wt": wt_full, "pack0": _to_mm(p0)}
        if use_bias:
            # bias columns in natural chunk-major order (bias_col indexes)
            bcr = np.empty((1, TOTAL_COLS + NG_C + B), np.float32)
            bcr[0, :TOTAL_COLS] = bc.reshape(-1) * np.float32(
                MM_SCALE * MM_SCALE)
            bcr[0, TOTAL_COLS:TOTAL_COLS + NG_C] = bg_all[g_j] * np.float32(
                MM_SCALE * MM_SCALE)
            bcr[0, TOTAL_COLS + NG_C:] = 1.0
            m["bias_m"] = _round_fp32r(np.ascontiguousarray(bcr))
        in_maps.append(m)
    return in_maps


def _build_fast(nc):
    """Cache a jitted executor for this nc so repeat kernel() calls skip
    retracing/recompiling (mirrors bass2jax.run_bass_via_pjrt)."""
    import jax
    from concourse import bass2jax
    from concourse.bass2jax import _bass_exec_p, partition_id_tensor
    from jax.experimental.shard_map import shard_map
    from jax.sharding import Mesh, PartitionSpec

    bass2jax.install_neuronx_cc_hook()
    partition_name = nc.partition_id_tensor.name if nc.partition_id_tensor else None
    in_names, out_names, out_avals, zero_outs = [], [], [], []
    for alloc in nc.m.functions[0].allocations:
        if not isinstance(alloc, mybir.MemoryLocationSet):
            continue
        name = alloc.memorylocations[0].name
        if alloc.kind == "ExternalInput":
            if name != partition_name:
                in_names.append(name)
        elif alloc.kind == "ExternalOutput":
            out_names.append(name)
            shape = tuple(alloc.tensor_shape)
            dtype = mybir.dt.np(alloc.dtype)
            out_avals.append(jax.core.ShapedArray(shape, dtype))
            zero_outs.append(np.zeros(shape, dtype))
    n_params = len(in_names)
    all_in = list(in_names) + list(out_names)
    if partition_name is not None:
        all_in.append(partition_name)

    def _body(*args):
        ops = list(args)
        if partition_name is not None:
            ops.append(partition_id_tensor())
        return tuple(
            _bass_exec_p.bind(
                *ops,
                out_avals=tuple(out_avals),
                in_names=tuple(all_in),
                out_names=tuple(out_names),
                lowering_input_output_aliases=(),
                sim_require_finite=True,
                sim_require_nnan=True,
                nc=nc,
            )
        )

    devices = jax.devices()[:NCORES]
    mesh = Mesh(np.asarray(devices), ("core",))
    spec = PartitionSpec("core")
    fn = jax.jit(
        shard_map(
            _body, mesh=mesh,
            in_specs=(spec,) * (n_params + len(out_names)),
            out_specs=(spec,) * len(out_names), check_rep=False,
        ),
        keep_unused=True,
    )
    _fast[id(nc)] = (fn, in_names, out_names, out_avals, zero_outs, mesh, spec)


def _run_cached(nc, in_maps):
    fn, in_names, out_names, out_avals, zero_outs, mesh, spec = _fast[id(nc)]
    concat_in = [
        np.concatenate([np.asarray(in_maps[c][nm]) for c in range(NCORES)], axis=0)
        for nm in in_names
    ]
    concat_zero = [
        np.zeros((NCORES * z.shape[0], *z.shape[1:]), z.dtype) for z in zero_outs
    ]
    outs = fn(*concat_in, *concat_zero)
    return [
        {
            nm: np.asarray(outs[i]).reshape(NCORES, *out_avals[i].shape)[c]
            for i, nm in enumerate(out_names)
        }
        for c in range(NCORES)
    ]


def kernel(last_hidden_state, param_w, param_b, points):
    global _last_results, _last_nc
    from concourse.bass_utils import run_bass_kernel_spmd

    lhs = np.asarray(last_hidden_state, dtype=np.float32)
    W = np.ascontiguousarray(np.asarray(param_w, dtype=np.float32))
    bias_vec = np.asarray(param_b, dtype=np.float32)
    pts = np.asarray(points)

    use_bias = bool(np.any(bias_vec))
    ht = np.ascontiguousarray(lhs[:, -1, :].T)  # (D, B)
    in_maps = _prep_core_inputs(W, bias_vec, pts, ht, use_bias)

    nc = _get_nc(use_bias=use_bias)
    _last_nc = nc
    if id(nc) in _fast:
        results = _run_cached(nc, in_maps)
    else:
        res = run_bass_kernel_spmd(nc, in_maps, core_ids=list(range(NCORES)))
        _last_results = res
        results = res.results
        _build_fast(nc)

    _, _, _, scol_chunk = _plan()
    sacc = np.zeros((B, GC0), np.float64)
    stail = np.zeros((B,), np.float64)
    for r_ in results:
        sacc += r_["sg_out"][:, :GC0].astype(np.float64)
        stail += r_["sg_out"][:, TEXP0:TEXP0 + STRIPW].astype(
            np.float64).sum(axis=1)
    s32 = np.zeros((B, NCHUNK), np.float64)
    for col, ch in enumerate(scol_chunk):
        s32[:, ch] += sacc[:, col]
    s32[:, NCHUNK - 1] += stail
    s0, s1 = s32[:, :R], s32[:, R:]
    norm_const = (s0 * s1).sum(axis=1)

    g_all = np.stack(
        [r_["sg_out"][:, GC0:GC0 + NG_C].astype(np.float64) for r_ in results],
        axis=1,
    ).reshape(B, NG_ALL)
    bi = np.arange(B)[:, None]
    ri = np.arange(R)[None, :]
    g0 = g_all[bi, ri * B + bi]
    g1 = g_all[bi, (R + ri) * B + bi]
    p_eval = (g0 * g1).sum(axis=1)
    return p_eval.astype(np.float32), norm_const.astype(np.float32)
